# revision 39
# baseline (speedup 1.0000x reference)
"""CARC attention processor kernel for 8 Trainium2 NeuronCores.

Sharding: data-parallel over the fused B*H axis. 80 heads / 8 cores =
10 heads per core; each core owns one batch (bi = core//2) and one
10-head group (g = core%2). Projection weights are column/row-sliced
per head group; the KV bank is sliced per core. Each core emits a
partial output projection over its 640 channels; the host sums the two
partials per batch and adds the bias.

Device algorithm per core:
  - qT/kT projections in transposed layout [64*heads, L] (Dh on
    partitions) so scores can contract over Dh directly.
  - q is evacuated into TWO zero-padded tiles per pair (qz0 = head0's
    qT on partitions 0:64 + zeros below, qz1 = zeros + head1's qT on
    64:128) so every scores matmul runs the full 128-partition array
    with the SAME kT stationary operand for both heads: the zero rows
    annihilate the other head's contribution, N-streaming cost is
    unchanged, and no quadrant/tile_position weight switches stall the
    PE (those cost ~90ns per matmul in the v1 kernel).
  - v projection lands in [keys, head*128] layout where each head's 64
    value columns are followed by 64 ones columns: the ctx matmul
    lhsT [128 keys, v|ones] then yields ctxT in PSUM rows 0:64 and the
    softmax denominator (replicated x64) in rows 64:128.
  - exp with the 1/sqrt(Dh) scale fused into the ACT activation (no
    max subtraction: |scores| < ~6 so exp is safe in fp32).
  - softmax normalization: pairs 0-3 use the stock DVE reciprocal off
    the critical path; the last pair (which gates the output
    projection) uses a raw ACT Reciprocal activation (accurate enough
    for a softmax denominator) plus chunked DVE multiplies, and the
    out-projection accumulates pairs 0-3 ahead so only the final
    stop-matmul waits on the normalize.
  - output projection contracts head pairs (K=128) of ctxT against
    row-slices of Wo, accumulating 5 pair-matmuls in PSUM.

Startup is cc-chunk-pipelined: hsT/weight DMAs are split and ordered
by first use, and the first projections iterate cc-OUTER (v heads 0-1
+ pair-0 q/k accumulate across all PSUM tiles per arriving hsT chunk)
so the in-order PE queue never waits on a chunk that arrives after
one it needs first. Pair emission then interleaves scores(kc) with
ctx(kc-2) and the next pair's q/k projections at kc 5/7.
"""
from contextlib import ExitStack

import numpy as np

import concourse.bass as bass
import concourse.tile as tile
from concourse import bacc, mybir
from concourse import bass_utils

F32 = mybir.dt.float32
F16 = mybir.dt.float16
ActF = mybir.ActivationFunctionType

B, L, C, H, Dh = 4, 1024, 1280, 20, 64
NCORES = 8
HPC = 10               # heads per core
NP = HPC // 2          # head pairs per core
ALPHA = 0.8 * 0.6
LB = 256               # bank keys per head after 2x2 pooling
KEYS = L + LB          # 1280
KCH = KEYS // 128      # 10 key chunks
CC = C // 128          # 10 contraction chunks
LT = L // 128          # 8 query/row tiles


def _build():
    nc = bacc.Bacc("TRN2", target_bir_lowering=False, debug=False,
                   num_devices=NCORES)
    hsT_d = nc.dram_tensor("hsT", [C, L], F16, kind="ExternalInput")
    # wq/wk pre-arranged on host as [NP][128 part][CC][128 cols]
    wq_d = nc.dram_tensor("wq", [NP, 128, CC, 128], F16, kind="ExternalInput")
    wk_d = nc.dram_tensor("wk", [NP, 128, CC, 128], F16, kind="ExternalInput")
    # wv pre-arranged as [2 halves][128 part][CC][320 cols]
    wv_d = nc.dram_tensor("wv", [2, 128, CC, 320], F16, kind="ExternalInput")
    wo_d = nc.dram_tensor("wo", [HPC * Dh, C], F16, kind="ExternalInput")
    kbT_d = nc.dram_tensor("kbT", [HPC * Dh, LB], F16, kind="ExternalInput")
    vb_d = nc.dram_tensor("vb", [LB, HPC * Dh], F16, kind="ExternalInput")
    out_d = nc.dram_tensor("out", [L, C], F16, kind="ExternalOutput")

    with tile.TileContext(nc) as tc, ExitStack() as es:
        big = es.enter_context(tc.tile_pool(name="big", bufs=1))
        wst = es.enter_context(tc.tile_pool(name="wst", bufs=2))
        qkt = es.enter_context(tc.tile_pool(name="qkt", bufs=2))
        expp = es.enter_context(tc.tile_pool(name="expp", bufs=6))
        rcpp = es.enter_context(tc.tile_pool(name="rcpp", bufs=1))
        denp = es.enter_context(tc.tile_pool(name="denp", bufs=2))
        hs_es = ExitStack()
        hsp = hs_es.enter_context(tc.tile_pool(name="hsp", bufs=1))
        attn_es = ExitStack()
        pss = attn_es.enter_context(
            tc.tile_pool(name="pss", bufs=2, space="PSUM"))

        ctxT_sb = big.tile([128, NP, L], F16)
        v_sb = big.tile([128, KCH, HPC * 128], F16)
        v_heads = v_sb[:].rearrange("p c (h x) -> p c h x", x=128)
        ones32 = big.tile([128, HPC, Dh], F16)
        nc.vector.memset(ones32[:], 1.0)
        # persistent zero-padded q tiles (2 double-buffer sets x 2 heads);
        # singletons so the zero rows planted here survive across pairs
        qz_sets = [[big.tile([128, L], F16, tag=f"qz{s}{par}",
                             name=f"qz{s}{par}")
                    for par in range(2)] for s in range(2)]
        for s in range(2):
            nc.vector.memset(qz_sets[s][0][64:128, :], 0.0)
            nc.vector.memset(qz_sets[s][1][0:64, :], 0.0)

        hsT_sb = hsp.tile([128, CC, L], F16)
        # distinct tags: both halves stay resident (a shared bufs=1 slot
        # would WAR-chain wv1's DMA behind loop C's wv0 reads -> deadlock)
        wv0 = wst.tile([128, CC, 320], F16, tag="wv0", name="wv0", bufs=1)
        wv1 = wst.tile([128, CC, 320], F16, tag="wv1", name="wv1", bufs=1)
        wv_tiles = [wv0, wv1]
        wq0_sb = wst.tile([128, CC, 128], F16, tag="wq", name="wq0")
        wk0_sb = wst.tile([128, CC, 128], F16, tag="wk", name="wk0")

        # ---- input DMAs: hsT/wv0 round-robin by cc across the three
        # queues in first-use order (loop A consumes cc-ascending at
        # ~1.1us/chunk); wq0/wk0 halves head the other two queues ----
        dqs = [nc.sync, nc.gpsimd, nc.scalar]

        def hsdma(q, cc):
            q.dma_start(hsT_sb[:, cc, :],
                        hsT_d.ap()[cc * 128:(cc + 1) * 128, :])

        def wvdma(q, g, c0, c1, dst=None):
            q.dma_start((dst or wv_tiles[g])[:, c0:c1, :],
                        wv_d.ap()[g, :, c0:c1, :])

        # interleaved per-queue sequences in loop-A consumption order
        # (slot j eats hsT[j] + wv0[j], q/k weights lag two slots);
        # wv0 in cc-pair pieces so arrival tracks consumption. Chunk 0
        # alone is split by columns: the first matmuls touch only cols
        # 0:512, halving the bytes gating the very first matmul.
        nc.sync.dma_start(hsT_sb[:, 0, 0:256], hsT_d.ap()[0:128, 0:256])
        wvdma(nc.gpsimd, 0, 0, 1)
        nc.scalar.dma_start(wk0_sb[:, 0:2, :], wk_d.ap()[0, :, 0:2, :])
        nc.sync.dma_start(hsT_sb[:, 0, 256:L], hsT_d.ap()[0:128, 256:L])
        wvdma(nc.gpsimd, 0, 1, 2)
        nc.gpsimd.dma_start(wq0_sb[:, 0:2, :], wq_d.ap()[0, :, 0:2, :])
        hsdma(nc.scalar, 2)
        hsdma(nc.gpsimd, 1)
        wvdma(nc.sync, 0, 2, 4)
        hsdma(nc.sync, 3)
        nc.scalar.dma_start(wk0_sb[:, 2:5, :], wk_d.ap()[0, :, 2:5, :])
        nc.gpsimd.dma_start(wq0_sb[:, 2:5, :], wq_d.ap()[0, :, 2:5, :])
        wvdma(nc.scalar, 0, 4, 6)
        hsdma(nc.gpsimd, 4)
        hsdma(nc.scalar, 5)
        wvdma(nc.sync, 0, 6, 8)
        hsdma(nc.sync, 6)
        hsdma(nc.scalar, 7)
        wvdma(nc.gpsimd, 0, 8, 10)
        hsdma(nc.sync, 8)
        hsdma(nc.gpsimd, 9)
        nc.scalar.dma_start(wk0_sb[:, 5:10, :], wk_d.ap()[0, :, 5:10, :])
        nc.gpsimd.dma_start(wq0_sb[:, 5:10, :], wq_d.ap()[0, :, 5:10, :])
        nc.scalar.dma_start(wv1[:, 0:5, :], wv_d.ap()[1, :, 0:5, :])
        nc.scalar.dma_start(wv1[:, 5:10, :], wv_d.ap()[1, :, 5:10, :])

        # ones columns (DVE is idle at startup) + bank V columns
        for kc in range(KCH):
            nc.vector.tensor_copy(v_heads[:, kc, :, Dh:128], ones32[:])
        for j in range(LB // 128):
            nc.sync.dma_start(
                v_heads[:, LT + j, :, 0:Dh],
                vb_d.ap()[j * 128:(j + 1) * 128, :]
                .rearrange("p (h d) -> p h d", d=Dh))

        qts, kts = {}, {}

        def evac_q(m, pp):
            """PSUM q rows -> two zero-padded SBUF tiles (per head)."""
            qz0, qz1 = qz_sets[m % 2]
            nc.vector.tensor_copy(qz0[0:64, :], pp[0:64, :])
            nc.vector.tensor_copy(qz1[64:128, :], pp[64:128, :])
            qts[m] = (qz0, qz1)

        def evac_k(m, pp):
            kt = qkt.tile([128, KEYS], F16, tag="kT", name=f"kT{m}")
            nc.vector.tensor_copy(kt[:, 0:L], pp[:])
            nc.sync.dma_start(kt[:, L:KEYS],
                              kbT_d.ap()[m * 128:(m + 1) * 128, :])
            kts[m] = kt

        def emit_proj_part(m, which):
            """One of the q/k projections for pair m (m >= 1)."""
            if which == "q":
                w_d, wtag = wq_d, "wq"
            else:
                w_d, wtag = wk_d, "wk"
            w_sb = wst.tile([128, CC, 128], F16, tag=wtag, name=f"{wtag}{m}")
            (nc.sync if which == 'q' else nc.scalar).dma_start(
                w_sb[:], w_d.ap()[m])
            pp = pss.tile([128, L], F32, tag="ps", name=f"p{wtag}{m}")
            for qh in range(2):
                for cc in range(CC):
                    nc.tensor.matmul(
                        pp[:, qh * 512:(qh + 1) * 512],
                        w_sb[:, cc, :],
                        hsT_sb[:, cc, qh * 512:(qh + 1) * 512],
                        start=(cc == 0), stop=(cc == CC - 1))
            if which == "q":
                evac_q(m, pp)
            else:
                evac_k(m, pp)

        # ---- startup: cc-outer so the in-order PE queue consumes hsT
        # chunks in arrival order (v heads 0-1 of both halves + all of
        # pair 0's q/k), then the remaining v row-tiles cc-inner ----
        psv_es = ExitStack()
        psv = psv_es.enter_context(
            tc.tile_pool(name="psv", bufs=4, space="PSUM"))

        def vproj_evac(pv, g, lt):
            nc.vector.tensor_copy(
                v_heads[:, lt, g * 5:(g + 1) * 5, 0:Dh],
                pv[:, 0:320].rearrange("p (h d) -> p h d", d=Dh))

        ppq = pss.tile([128, L], F32, tag="ps", name="pq0")
        ppk = pss.tile([128, L], F32, tag="ps", name="pk0")
        pvA = {lt: psv.tile([128, 512], F32, tag="pv", name=f"pva{lt}")
               for lt in range(4)}
        def qkmm(cc):
            for pp, w_sb in ((ppq, wq0_sb), (ppk, wk0_sb)):
                for qh in range(2):
                    nc.tensor.matmul(
                        pp[:, qh * 512:(qh + 1) * 512],
                        w_sb[:, cc, :],
                        hsT_sb[:, cc, qh * 512:(qh + 1) * 512],
                        start=(cc == 0), stop=(cc == CC - 1))

        # q/k lag the v chunks by one slot so the first matmuls only
        # need hsT[0] + wv0[0] and the w weights get DMA slack
        for cc in range(CC):
            for lt in range(4):
                nc.tensor.matmul(
                    pvA[lt][:, 0:320],
                    hsT_sb[:, cc, lt * 128:(lt + 1) * 128],
                    wv0[:, cc, :],
                    start=(cc == 0), stop=(cc == CC - 1))
            if cc >= 1:
                qkmm(cc - 1)
        qkmm(CC - 1)
        evac_q(0, ppq)
        evac_k(0, ppk)
        for lt in range(4):
            vproj_evac(pvA[lt], 0, lt)
        pvB = {lt: psv.tile([128, 512], F32, tag="pv", name=f"pvb{lt}")
               for lt in range(4)}
        for cc in range(CC):
            for lt in range(4):
                nc.tensor.matmul(
                    pvB[lt][:, 0:320],
                    hsT_sb[:, cc, lt * 128:(lt + 1) * 128],
                    wv1[:, cc, :],
                    start=(cc == 0), stop=(cc == CC - 1))
        for lt in range(4):
            vproj_evac(pvB[lt], 1, lt)
        for lt in range(4, LT):
            for g in range(2):
                pv = psv.tile([128, 512], F32, tag="pv", name=f"pvc{g}_{lt}")
                for cc in range(CC):
                    nc.tensor.matmul(
                        pv[:, 0:320],
                        hsT_sb[:, cc, lt * 128:(lt + 1) * 128],
                        wv_tiles[g][:, cc, :],
                        start=(cc == 0), stop=(cc == CC - 1))
                vproj_evac(pv, g, lt)
        psv_es.close()
        psc = attn_es.enter_context(
            tc.tile_pool(name="psc", bufs=2, space="PSUM"))

        ctxps_exp = {}

        def emit_scores(m, kc):
            qz = qts[m]
            ss = []
            for par in range(2):
                s = pss.tile([128, L], F32, tag="ps", name=f"s{m}_{kc}_{par}")
                ss.append(s)
            for par in range(2):
                for n0 in (0, 512):
                    nc.tensor.matmul(
                        ss[par][:, n0:n0 + 512],
                        kts[m][:, kc * 128:(kc + 1) * 128],
                        qz[par][:, n0:n0 + 512],
                        start=True, stop=True)
            for par in range(2):
                e = expp.tile([128, L], F16, tag="e", name=f"e{m}_{kc}_{par}")
                nc.scalar.activation(e[:], ss[par][:], ActF.Exp, scale=0.125)
                ctxps_exp[(m, kc, par)] = e

        def emit_ctx(m, kc, ctxps):
            for par in range(2):
                e = ctxps_exp.pop((m, kc, par))
                for n0 in (0, 512):
                    nc.tensor.matmul(
                        ctxps[par][:, n0:n0 + 512],
                        v_sb[:, kc, (2 * m + par) * 128:
                             (2 * m + par + 1) * 128],
                        e[:, n0:n0 + 512],
                        start=(kc == 0), stop=(kc == KCH - 1))

        wo_tiles = []
        for m in range(NP):
            ctxps = [psc.tile([128, L], F32, tag="ctx", name=f"ctx{m}_{par}")
                     for par in range(2)]
            # kc 0-1 of pairs >= 1 were pre-emitted across the previous
            # pair's boundary (below) to keep the exp pipeline fed
            for kc in range(2 if m else 0, KCH):
                emit_scores(m, kc)
                if m < NP - 1:
                    # ctx chunks are rearranged so TWO groups (1.7us of
                    # PE work) sit between a scores group and the next
                    # projection's PSUM allocation, covering the ~2.2us
                    # exp latency that otherwise stalls the allocation
                    if kc == 5:
                        emit_ctx(m, 3, ctxps)
                        emit_ctx(m, 4, ctxps)
                        emit_proj_part(m + 1, "q")
                    elif kc == 7:
                        emit_ctx(m, 5, ctxps)
                        emit_ctx(m, 6, ctxps)
                        emit_proj_part(m + 1, "k")
                    elif kc == 9:
                        emit_ctx(m, 7, ctxps)
                    elif 2 <= kc <= 4:
                        emit_ctx(m, kc - 2, ctxps)
                elif kc >= 2:
                    emit_ctx(m, kc - 2, ctxps)
            if m < NP - 1:
                emit_scores(m + 1, 0)
                emit_ctx(m, KCH - 2, ctxps)
                emit_scores(m + 1, 1)
                emit_ctx(m, KCH - 1, ctxps)
            else:
                # hsT no longer needed; free its SBUF before wo loads
                hs_es.close()
                wop = es.enter_context(tc.tile_pool(name="wop", bufs=1))
                for p in range(NP):
                    wo_sb = wop.tile([128, C], F16, tag=f"wo{p}")
                    (nc.sync if p % 2 == 0 else nc.scalar).dma_start(
                        wo_sb[:], wo_d.ap()[p * 128:(p + 1) * 128, :])
                    wo_tiles.append(wo_sb)
                emit_ctx(m, KCH - 2, ctxps)
                emit_ctx(m, KCH - 1, ctxps)
            den = denp.tile([128, L], F32, tag="den", name=f"den{m}")
            rc = rcpp.tile([128, L], F32, tag="rc", name=f"rc{m}")
            if m < NP - 1:
                # raw evacuation first (releases the PSUM ctx slots fast),
                # then normalize in place, off the critical path
                for par in range(2):
                    sl = slice(64 * par, 64 * par + 64)
                    nc.vector.tensor_copy(ctxT_sb[sl, m, :],
                                          ctxps[par][0:64, :])
                    nc.vector.tensor_copy(den[sl, :], ctxps[par][64:128, :])
                nc.vector.reciprocal(rc[:], den[:])
                nc.vector.tensor_mul(
                    ctxT_sb[:, m, :], ctxT_sb[:, m, :], rc[:])
            else:
                # last pair gates the output projection. 1/den on the
                # idle ACT engine via a raw InstActivation(Reciprocal) —
                # bass's wrapper refuses it for accuracy, but a softmax
                # denominator only needs ~1e-3 relative accuracy (the
                # error scales the output uniformly). One table-set load
                # at first use; each [64,512] recip is then ~0.4us vs
                # 3.4us for the stock DVE RECIPROCAL, and ln/exp's 6
                # table switches are avoided. DVE only multiplies.
                def act_recip(out_ap, in_ap):
                    se = nc.scalar
                    ins_ = [se.lower_ap(in_ap)] + [
                        mybir.ImmediateValue(dtype=mybir.dt.float32,
                                             value=v)
                        for v in (0.0, 1.0, 0.0)]
                    se.add_instruction(mybir.InstActivation(
                        name=se.bass.get_next_instruction_name(),
                        func=ActF.Reciprocal,
                        ins=ins_, outs=[se.lower_ap(out_ap)]))

                for half in range(2):
                    hsl = slice(half * 512, (half + 1) * 512)
                    act_recip(rc[64:128, hsl], ctxps[0][64:128, hsl])
                    act_recip(den[64:128, hsl], ctxps[1][64:128, hsl])
                    for i in range(2):
                        qs = slice(half * 512 + i * 256,
                                   half * 512 + (i + 1) * 256)
                        nc.vector.tensor_mul(ctxT_sb[0:64, m, qs],
                                             ctxps[0][0:64, qs],
                                             rc[64:128, qs])
                        nc.vector.tensor_mul(ctxT_sb[64:128, m, qs],
                                             ctxps[1][0:64, qs],
                                             den[64:128, qs])

        # ---- output projection (psum from the pss pool: its slots free
        # right after the last exp, ~8us before psc's). Pairs 0-3 are
        # accumulated AHEAD of the stop matmul so only pair 4's final
        # matmul waits on the last pair's normalization chunks ----
        with tc.tile_pool(name="outp", bufs=8) as outp:
            specs = [(qt, n0, nsz) for qt in range(LT)
                     for n0, nsz in ((0, 512), (512, 512), (1024, 256))]
            pend = []

            def finish(sp):
                idx, sub, qt, n0, nsz = sp
                nc.tensor.matmul(
                    sub[:, 0:nsz],
                    ctxT_sb[:, NP - 1, qt * 128:(qt + 1) * 128],
                    wo_tiles[NP - 1][:, n0:n0 + nsz],
                    start=False, stop=True)
                ob = outp.tile([128, 512], F16, tag="ob",
                               name=f"ob{qt}_{n0}")
                # alternate evac engines: DVE is near-saturated with the
                # normalize muls; ACT is idle in this phase (and unlike
                # GpSimd it can read PSUM; Copy is in every table set)
                if idx % 3 == 2:
                    nc.scalar.copy(ob[:, 0:nsz], sub[:, 0:nsz])
                else:
                    nc.vector.tensor_copy(ob[:, 0:nsz], sub[:, 0:nsz])
                # tail DMAs go to scalar (its ACT copies are done by
                # then); earlier ones alternate sync/gpsimd
                q = ([nc.sync, nc.gpsimd, nc.scalar][idx % 3] if idx >= 18
                     else (nc.sync if idx % 2 == 0 else nc.gpsimd))
                q.dma_start(
                    out_d.ap()[qt * 128:(qt + 1) * 128, n0:n0 + nsz],
                    ob[:, 0:nsz])

            # two outproj tiles share one [128,1024] pss buffer (each
            # writes <=512 cols = 1 bank): doubles the effective PSUM
            # rotation depth so allocations stop waiting on evac tails
            po_cur = None
            for idx, (qt, n0, nsz) in enumerate(specs):
                if idx % 2 == 0:
                    po_cur = pss.tile([128, L], F32, tag="ps",
                                      name=f"po{idx}")
                sub = po_cur[:, (idx % 2) * 512:(idx % 2) * 512 + 512]
                for p in range(NP - 1):
                    nc.tensor.matmul(
                        sub[:, 0:nsz],
                        ctxT_sb[:, p, qt * 128:(qt + 1) * 128],
                        wo_tiles[p][:, n0:n0 + nsz],
                        start=(p == 0), stop=False)
                pend.append((idx, sub, qt, n0, nsz))
                if len(pend) == 2:
                    finish(pend.pop(0))
            for sp in pend:
                finish(sp)
        attn_es.close()
    nc.compile()
    return nc


_NC = None


def _get_nc():
    global _NC
    if _NC is None:
        _NC = _build()
    return _NC


def _prep_in_maps(hidden_states, Wq, Wk, Wv, Wo, K_bg, V_bg):
    hs = np.asarray(hidden_states, np.float32)
    Wq, Wk, Wv, Wo = (np.asarray(w, np.float32) for w in (Wq, Wk, Wv, Wo))
    K_bg = np.asarray(K_bg, np.float32)
    V_bg = np.asarray(V_bg, np.float32)

    hsT = [np.ascontiguousarray(hs[bi].T).astype(np.float16)
           for bi in range(B)]

    def lay_qk(w, g):  # [1280, 640] slice -> [NP, 128, CC, 128]
        sl = w[:, g * 640:(g + 1) * 640]           # [C, 640]
        a = sl.reshape(CC, 128, NP, 128)           # (cc, p, m, n)
        return np.ascontiguousarray(a.transpose(2, 1, 0, 3)).astype(np.float16)

    def lay_wv(w, g):  # [1280, 640] slice -> [2, 128, CC, 320]
        sl = w[:, g * 640:(g + 1) * 640]
        a = sl.reshape(CC, 128, 2, 320)            # (cc, p, gg, n)
        return np.ascontiguousarray(a.transpose(2, 1, 0, 3)).astype(np.float16)

    wq_s = [lay_qk(Wq, g) for g in range(2)]
    wk_s = [lay_qk(Wk, g) for g in range(2)]
    wv_s = [lay_wv(Wv, g) for g in range(2)]
    wo_s = [Wo[g * 640:(g + 1) * 640, :].astype(np.float16) for g in range(2)]

    def pool_bank(x):  # [10, 1024, 64] -> [10, 256, 64], fp16 round + alpha
        x = x.astype(np.float16).astype(np.float32)
        x = x.reshape(HPC, 16, 2, 16, 2, Dh).mean(axis=(2, 4))
        return (ALPHA * x).reshape(HPC, LB, Dh)

    kb_s, vb_s = [], []
    for base in (0, 10, 20, 30):
        kb = pool_bank(K_bg[base:base + HPC])
        vb = pool_bank(V_bg[base:base + HPC])
        kb_s.append(kb.transpose(0, 2, 1).reshape(HPC * Dh, LB).astype(np.float16))
        vb_s.append(vb.transpose(1, 0, 2).reshape(LB, HPC * Dh).astype(np.float16))

    in_maps = []
    for c in range(NCORES):
        bi, g = c // 2, c % 2
        bank = (20 * bi + 10 * g) % 40 // 10
        in_maps.append({
            "hsT": hsT[bi], "wq": wq_s[g], "wk": wk_s[g], "wv": wv_s[g],
            "wo": wo_s[g], "kbT": kb_s[bank], "vb": vb_s[bank],
        })
    return in_maps


def _run(in_maps, **kwargs):
    return bass_utils.run_bass_kernel_spmd(
        _get_nc(), in_maps, core_ids=list(range(NCORES)), **kwargs)


def kernel(hidden_states, Wq, Wk, Wv, Wo, bo, K_bg, V_bg):
    in_maps = _prep_in_maps(hidden_states, Wq, Wk, Wv, Wo, K_bg, V_bg)
    res = _run(in_maps)
    bo = np.asarray(bo, np.float32)
    out = np.empty((B, L, C), np.float32)
    for bi in range(B):
        out[bi] = (res.results[2 * bi]["out"].astype(np.float32)
                   + res.results[2 * bi + 1]["out"].astype(np.float32)
                   + bo[None, :])
    return out
